# revision 1
# baseline (speedup 1.0000x reference)
"""Trainium2 Bass kernel for nn_Attention_712964571585.

Grouped multi-head attention with RoPE and null-KV, B=4 G=2 N=2048 D=512
H=8 DH=64. Sharded data-parallel over B*G = 8 NeuronCores (core c handles
b=c//2, g=c%2); each core runs the full per-(b,g) attention block:

  - QKV projections as bf16 matmuls with the RoPE rotate-half permutation
    folded into pre-rotated weight copies (q_rot = (R Wq) x), rope combine
    (q*cos + q_rot*sin) on the vector engine; the 1/sqrt(dh) scale is
    folded into the q-side cos/sin tables.
  - Attention in S^T layout ([k, q], k on partitions): softmax without
    max-subtraction (logits are O(6); exp in fp32 PSUM is safe), exp on the
    scalar engine (PSUM->SBUF bf16, one [128, 1024] instruction per k-block
    covering both heads of a pair), softmax denominator via a ones-column
    appended to V (PV matmul with M=65), reciprocal broadcast across
    partitions on GPSIMD.
  - The null key/value is a 17th k-block (row 0 = null kv, rows 1..127
    masked via a per-partition exp bias of -60).
  - Output projection accumulated per head-pair into SBUF; y^T DMA'd out,
    host transposes back.
"""
import numpy as np
import ml_dtypes
import concourse.bass as bass
import concourse.mybir as mybir
from concourse import bacc
from concourse.tile import TileContext
from concourse.bass_utils import run_bass_kernel_spmd

F32 = mybir.dt.float32
BF16 = mybir.dt.bfloat16
AF = mybir.ActivationFunctionType
MULT = mybir.AluOpType.mult
ADD = mybir.AluOpType.add

B, G, N, D = 4, 2, 2048, 512
H, DH = 8, 64
DI = H * DH
DB = D // 128          # 4 d-blocks
EB = DI // 128         # 4 e-blocks (= head pairs)
NB = N // 128          # 16 n-blocks
NKB = NB + 1           # 17 k-blocks (incl null)
VW = DH + 1            # 65: V plus ones column
NEG = -60.0


def build_nc(QC=512, n_time_loops=1, variant=""):
    """Build the per-core Bass graph. QC: q-chunk size (one exp instruction
    covers both heads' S^T of one k-block: [128, 2*QC]).
    n_time_loops: repeat compute body (for timing)."""
    NQC = N // QC                   # q-chunks
    nc = bacc.Bacc(None)

    xT_d = nc.dram_tensor("xT", [D, N], BF16, kind="ExternalInput")
    wq_d = nc.dram_tensor("wq", [D, DI], BF16, kind="ExternalInput")
    wqr_d = nc.dram_tensor("wqr", [D, DI], BF16, kind="ExternalInput")
    wk_d = nc.dram_tensor("wk", [D, DI], BF16, kind="ExternalInput")
    wkr_d = nc.dram_tensor("wkr", [D, DI], BF16, kind="ExternalInput")
    wv_d = nc.dram_tensor("wv", [D, DI], BF16, kind="ExternalInput")
    wout_d = nc.dram_tensor("wout", [DI, D], BF16, kind="ExternalInput")
    cosq_d = nc.dram_tensor("cosq", [128, N], BF16, kind="ExternalInput")
    sinq_d = nc.dram_tensor("sinq", [128, N], BF16, kind="ExternalInput")
    cosk_d = nc.dram_tensor("cosk", [128, N], BF16, kind="ExternalInput")
    sink_d = nc.dram_tensor("sink", [128, N], BF16, kind="ExternalInput")
    nkp_d = nc.dram_tensor("nkp", [128, EB], F32, kind="ExternalInput")
    nvf_d = nc.dram_tensor("nvf", [1, DI], F32, kind="ExternalInput")
    nbias_d = nc.dram_tensor("nbias", [128, 1], F32, kind="ExternalInput")
    yT_d = nc.dram_tensor("out", [D, N], F32, kind="ExternalOutput")

    with TileContext(nc) as tc:
        with (
            tc.tile_pool(name="persist", bufs=1) as pp,
            tc.tile_pool(name="stage", bufs=2) as sp,
            tc.tile_pool(name="etile", bufs=3) as ep,
            tc.tile_pool(name="small", bufs=2) as mp,
            tc.tile_pool(name="actp", bufs=2) as ap_pool,
            tc.tile_pool(name="psA", bufs=2, space="PSUM") as psA,   # stAB [128,1024]
            tc.tile_pool(name="psB", bufs=1, space="PSUM") as psB,   # qkv [128,1024]
            tc.tile_pool(name="psC", bufs=1, space="PSUM") as psC,   # o_ps [65,QC] x2
            tc.tile_pool(name="drp", bufs=2, space="DRAM") as drp,
        ):
            # ---- persistent tiles (bf16 direct loads; x first for startup) ----
            xTb = [pp.tile([128, N], BF16, tag=f"xT{i}", name=f"xT{i}") for i in range(DB)]
            for i in range(DB):
                nc.sync.dma_start(xTb[i][:], xT_d[i * 128:(i + 1) * 128, :])
            wtiles = {}
            for wname, dram in (("wv", wv_d), ("wk", wk_d), ("wkr", wkr_d),
                                ("wq", wq_d), ("wqr", wqr_d), ("wout", wout_d)):
                bf = [pp.tile([128, 512], BF16, tag=f"{wname}b{i}", name=f"{wname}b{i}")
                      for i in range(4)]
                wtiles[wname] = bf
                for i in range(4):
                    nc.sync.dma_start(bf[i][:], dram[i * 128:(i + 1) * 128, :])
            cosq = pp.tile([128, N], BF16, tag="cosq", name="cosq")
            sinq = pp.tile([128, N], BF16, tag="sinq", name="sinq")
            cosk = pp.tile([128, N], BF16, tag="cosk", name="cosk")
            sink = pp.tile([128, N], BF16, tag="sink", name="sink")
            nc.sync.dma_start(cosk[:], cosk_d[:])
            nc.sync.dma_start(sink[:], sink_d[:])
            nc.sync.dma_start(cosq[:], cosq_d[:])
            nc.sync.dma_start(sinq[:], sinq_d[:])
            nbias = pp.tile([128, 1], F32, tag="nbias", name="nbias")
            nc.sync.dma_start(nbias[:], nbias_d[:])
            nkp = pp.tile([128, EB], F32, tag="nkp", name="nkp")
            nc.sync.dma_start(nkp[:], nkp_d[:])
            nvf = pp.tile([1, DI], F32, tag="nvf", name="nvf")
            nc.sync.dma_start(nvf[:], nvf_d[:])

            y_acc = [pp.tile([128, N], F32, tag=f"yac{i}", name=f"yac{i}")
                     for i in range(DB)]
            # ---- per-head-pair persistent activation tiles ----
            qT, kT, qTB, kTB, OT = {}, {}, {}, {}, {}
            # V': [128, H*VW] per n-block (17th = null)
            Vp = [pp.tile([128, H * VW], BF16, tag=f"Vp{nb}", name=f"Vp{nb}") for nb in range(NKB)]

            def emit_vproj(nbs, with_null):
                for nb in nbs:
                    v_ps = psB.tile([128, 1024], F32, tag="qkv", name="qkv")[:, :512]
                    for db in range(DB):
                        nc.tensor.matmul(
                            v_ps, xTb[db][:, nb * 128:(nb + 1) * 128],
                            wtiles["wv"][db][:],
                            start=(db == 0), stop=(db == DB - 1))
                    vt = Vp[nb][:].rearrange("p (h w) -> p h w", h=H)
                    nc.vector.memset(vt[:, :, DH:VW], 1.0)
                    nc.vector.tensor_copy(
                        vt[:, :, 0:DH],
                        v_ps.rearrange("p (h j) -> p h j", h=H))
                if with_null:
                    vt = Vp[NB][:].rearrange("p (h w) -> p h w", h=H)
                    nc.vector.memset(Vp[NB][:], 0.0)
                    nc.vector.memset(vt[:, :, DH:VW], 1.0)
                    nc.vector.tensor_copy(
                        vt[0:1, :, 0:DH],
                        nvf[:].rearrange("p (h j) -> p h j", h=H))

            import contextlib
            loop_ctx = (tc.For_i(0, n_time_loops, 1) if n_time_loops > 1
                        else contextlib.nullcontext())
            with loop_ctx:
                for p in range(EB):
                    # ---- phase 1: q/k projection + rope for pair p ----
                    qT[p] = ap_pool.tile([128, N], BF16, tag="qT", name="qT")
                    kT[p] = ap_pool.tile([128, N + 128], BF16, tag="kT", name="kT")
                    qTB[p] = ap_pool.tile([64, N], BF16, tag="qTB", name="qTB")
                    kTB[p] = ap_pool.tile([64, N + 128], BF16, tag="kTB", name="kTB")
                    OT[p] = ap_pool.tile([128, N], BF16, tag="OT", name="OT")
                    e0 = p * 128
                    for ncki in range(N // 512):
                        for which, wmain, wrot, ctab, stab, dstT in (
                            ("k", wtiles["wk"], wtiles['wkr'], cosk, sink, kT[p]),
                            ("q", wtiles["wq"], wtiles['wqr'], cosq, sinq, qT[p]),
                        ):
                            s = slice(ncki * 512, (ncki + 1) * 512)
                            pr_ps = psB.tile([128, 1024], F32, tag="qkv", name="qkv")
                            m_ps, r_ps = pr_ps[:, :512], pr_ps[:, 512:]
                            for db in range(DB):
                                nc.tensor.matmul(
                                    m_ps, wmain[db][:, e0:e0 + 128],
                                    xTb[db][:, s],
                                    start=(db == 0), stop=(db == DB - 1))
                            for db in range(DB):
                                nc.tensor.matmul(
                                    r_ps, wrot[db][:, e0:e0 + 128],
                                    xTb[db][:, s],
                                    start=(db == 0), stop=(db == DB - 1))
                            m1 = mp.tile([128, 512], F32, tag="ropea", name="ropea")
                            m2 = mp.tile([128, 512], F32, tag="ropeb", name="ropeb")
                            nc.vector.tensor_tensor(m1[:], m_ps, ctab[:, s], MULT)
                            nc.vector.tensor_tensor(m2[:], r_ps, stab[:, s], MULT)
                            nc.vector.tensor_tensor(dstT[:, s], m1[:], m2[:], ADD)
                    # null key column
                    nc.vector.memset(kT[p][:, N:N + 128], 0.0)
                    nc.vector.tensor_copy(kT[p][:, N:N + 1], nkp[:, p:p + 1])
                    # head-B operands copied to base-0 tiles (base-64 matmul
                    # operands measure slower on HW)
                    nc.sync.dma_start(qTB[p][:], qT[p][64:128, :])
                    nc.sync.dma_start(kTB[p][:], kT[p][64:128, :])

                    if p == 0:
                        emit_vproj(range(0, 6), with_null=False)

                    # ---- phase 2: attention for pair p ----
                    for qc in range(NQC):
                        qs = slice(qc * QC, (qc + 1) * QC)
                        o_A = psC.tile([65, QC], F32, tag="opsA", name="opsA")
                        o_B = psC.tile([65, QC], F32, tag="opsB", name="opsB")
                        for kb in range(NKB):
                            if p == 0 and qc == 0 and 6 <= kb + 6 <= NB:
                                vb = kb + 6
                                if vb < NB:
                                    emit_vproj([vb], with_null=False)
                                else:
                                    emit_vproj([], with_null=True)
                            ks = slice(kb * 128, (kb + 1) * 128)
                            stAB = psA.tile([128, 2 * QC], F32, tag="stAB", name="stAB")
                            nc.tensor.matmul(
                                stAB[:, 0:QC], kT[p][0:64, ks],
                                qT[p][0:64, qs], start=True, stop=True)
                            nc.tensor.matmul(
                                stAB[:, QC:2 * QC], kTB[p][:, ks],
                                qTB[p][:, qs], start=True, stop=True)
                            bias = nbias[:, 0:1] if kb == NKB - 1 else 0.0
                            eAB = ep.tile([128, 2 * QC], BF16, tag="eAB", name="eAB")
                            nc.scalar.activation(eAB[:], stAB[:], AF.Exp, bias=bias)
                            vA = Vp[kb][:, (2 * p) * VW:(2 * p + 1) * VW]
                            vB = Vp[kb][:, (2 * p + 1) * VW:(2 * p + 2) * VW]
                            nc.tensor.matmul(
                                o_A, vA, eAB[:, 0:QC],
                                start=(kb == 0), stop=(kb == NKB - 1))
                            nc.tensor.matmul(
                                o_B, vB, eAB[:, QC:2 * QC],
                                start=(kb == 0), stop=(kb == NKB - 1))
                        # copy psum -> sbuf immediately so psC frees fast;
                        # normalize chain then runs off the critical path
                        ocp = mp.tile([65, 2 * QC], F32, tag="ocp", name="ocp")
                        nc.vector.tensor_copy(ocp[:, 0:QC], o_A[:])
                        nc.vector.tensor_copy(ocp[:, QC:2 * QC], o_B[:])
                        rch = mp.tile([128, 2 * QC], F32, tag="reciph", name="reciph")
                        nc.vector.reciprocal(rch[64:65, :], ocp[64:65, :])
                        rb = mp.tile([64, 2 * QC], F32, tag="rbcast", name="rbcast")
                        scr = drp.tile([1, 2 * QC], F32, tag="scr", name="scr")
                        nc.sync.dma_start(scr[:], rch[64:65, :])
                        nc.sync.dma_start(rb[:], scr[:].to_broadcast([64, 2 * QC]))
                        nc.vector.tensor_tensor(
                            OT[p][0:64, qs], ocp[0:64, 0:QC], rb[:, 0:QC], MULT)
                        oBt = mp.tile([64, QC], BF16, tag="oBtmp", name="oBtmp")
                        nc.vector.tensor_tensor(
                            oBt[:], ocp[0:64, QC:2 * QC], rb[:, QC:2 * QC], MULT)
                        nc.sync.dma_start(OT[p][64:128, qs], oBt[:])


                    # partial out-projection contribution of pair p
                    for dbk in range(DB):
                        for ncki in range(N // 512):
                            s = slice(ncki * 512, (ncki + 1) * 512)
                            tg = "opsA" if (dbk * 4 + ncki) % 2 == 0 else "opsB"
                            half = psC.tile([128, 512], F32, tag=tg, name=tg)[:, :]
                            nc.tensor.matmul(
                                half, wtiles["wout"][p][:, dbk * 128:(dbk + 1) * 128],
                                OT[p][:, s], start=True, stop=True)
                            if p == 0:
                                nc.vector.tensor_copy(y_acc[dbk][:, s], half)
                            else:
                                nc.vector.tensor_tensor(
                                    y_acc[dbk][:, s], y_acc[dbk][:, s], half, ADD)
                            if p == EB - 1:
                                nc.sync.dma_start(
                                    yT_d[dbk * 128:(dbk + 1) * 128, s],
                                    y_acc[dbk][:, s])

    return nc


# ---------------- host-side prep ----------------

def rot_weight(W):
    Wr = np.empty_like(W)
    for h in range(H):
        b0 = h * DH
        Wr[b0:b0 + 32] = -W[b0 + 32:b0 + 64]
        Wr[b0 + 32:b0 + 64] = W[b0:b0 + 32]
    return Wr


def host_prep_shared(Wq, Wkv, Wout, null_kv, rot_q, rot_k):
    """Per-group (g) tensors shared by all cores of that group."""
    import ml_dtypes
    bf = ml_dtypes.bfloat16
    scale = DH ** -0.5
    shared = []
    for g in range(G):
        d = {}
        Wqg = np.asarray(Wq[g], np.float32)
        Wk_, Wv_ = np.asarray(Wkv[g][:DI], np.float32), np.asarray(Wkv[g][DI:], np.float32)
        d["wq"] = np.ascontiguousarray(Wqg.T).astype(bf)
        d["wqr"] = np.ascontiguousarray(rot_weight(Wqg).T).astype(bf)
        d["wk"] = np.ascontiguousarray(Wk_.T).astype(bf)
        d["wkr"] = np.ascontiguousarray(rot_weight(Wk_).T).astype(bf)
        d["wv"] = np.ascontiguousarray(Wv_.T).astype(bf)
        d["wout"] = np.ascontiguousarray(np.asarray(Wout[g], np.float32).T).astype(bf)
        cq = np.cos(rot_q).T.astype(np.float32) * scale
        sq = np.sin(rot_q).T.astype(np.float32) * scale
        ck = np.cos(rot_k).T.astype(np.float32)
        sk = np.sin(rot_k).T.astype(np.float32)
        d["cosq"] = np.ascontiguousarray(np.concatenate([cq, cq], 0)).astype(bf)
        d["sinq"] = np.ascontiguousarray(np.concatenate([sq, sq], 0)).astype(bf)
        d["cosk"] = np.ascontiguousarray(np.concatenate([ck, ck], 0)).astype(bf)
        d["sink"] = np.ascontiguousarray(np.concatenate([sk, sk], 0)).astype(bf)
        nk = np.asarray(null_kv[0, g, :, 0, :], np.float32)      # [H, DH]
        nv = np.asarray(null_kv[1, g, :, 0, :], np.float32)
        nkp = np.empty((128, EB), np.float32)
        for p in range(EB):
            nkp[0:64, p] = nk[2 * p]
            nkp[64:128, p] = nk[2 * p + 1]
        d["nkp"] = nkp
        d["nvf"] = np.ascontiguousarray(nv.reshape(1, DI))
        nbias = np.full((128, 1), NEG, np.float32)
        nbias[0, 0] = 0.0
        d["nbias"] = nbias
        shared.append(d)
    return shared


def host_in_maps(x, Wq, Wkv, Wout, null_kv, rot_q, rot_k):
    shared = host_prep_shared(Wq, Wkv, Wout, null_kv, rot_q, rot_k)
    in_maps = []
    for c in range(8):
        b, g = c // 2, c % 2
        m = dict(shared[g])
        import ml_dtypes
        m["xT"] = np.ascontiguousarray(np.asarray(x[b, g], np.float32).T).astype(ml_dtypes.bfloat16)
        in_maps.append(m)
    return in_maps



_NC_CACHE = {}


def _get_nc():
    if "nc" not in _NC_CACHE:
        nc = build_nc()
        nc.finalize()
        _NC_CACHE["nc"] = nc
    return _NC_CACHE["nc"]


def kernel(x, Wq, Wkv, Wout, null_kv, rot_q, rot_k):
    """Full-input entry point: shards over 8 NeuronCores, returns [B,G,N,D]."""
    in_maps = host_in_maps(np.asarray(x), np.asarray(Wq), np.asarray(Wkv),
                           np.asarray(Wout), np.asarray(null_kv),
                           np.asarray(rot_q), np.asarray(rot_k))
    nc = _get_nc()
    res = run_bass_kernel_spmd(nc, in_maps, core_ids=list(range(8)))
    out = np.empty((B, G, N, D), np.float32)
    for c in range(8):
        b, g = c // 2, c % 2
        out[b, g] = np.asarray(res.results[c]["out"]).T
    return out

